# revision 1
# baseline (speedup 1.0000x reference)
"""Trainium2 Bass kernel for quantized int8 per-channel Conv2d.

Reference semantics (fp32):
  x_f = (x_int8 - 7) * 0.01                      # per-tensor dequant
  w_f = (w_int8 - zp[cout]) * scale[cout]        # per-channel dequant
  y   = round(conv2d_valid(x_f, w_f) + bias[cout])  -> int32

Exact-integer factorization used here:
  conv(x_f, w_f) = 0.01*scale[o] * S(o, p),  S = conv((x-7), (w-zp[o]))
(x-7) in [-135,120] and (w-zp) in [-137,137] are exact in bf16; products
accumulate exactly in fp32 PSUM (|S| << 2^24 for this data).  The final
affine + round happens in fp32 with the 1.5*2^23 magic-number trick,
which rounds half-to-even exactly like jnp.round.

Sharding: data-parallel over batch N=32 across 8 cores (4 images each);
weights/scales/zeropoints/bias replicated.
"""

import numpy as np

import concourse.bass as bass
import concourse.mybir as mybir
from concourse import bacc
from concourse.tile import TileContext
from concourse.bass_utils import run_bass_kernel_spmd

# Problem shapes (hardcoded per contract)
N, CIN, H, W = 32, 256, 56, 56
COUT, KH, KW = 256, 3, 3
HO, WO = H - KH + 1, W - KW + 1          # 54, 54
NCORES = 8
NPER = N // NCORES                        # images per core
HW = H * W                                # 3136
XPAD = HW + 4                             # pad: tap (2,2) of last chunk reads 2 past
CHUNK = 9 * WO                            # 486 = 9 output rows x 54 valid cols
NCHUNK = (HO * WO) // CHUNK               # 6
KT = (CIN // 128)                         # 2 cin tiles
MT = COUT // 128                          # 2 cout tiles
TAPS = KH * KW                            # 9
MAGIC = 12582912.0                        # 1.5 * 2**23  (fp32 RNE rounding trick)
B_CHUNK = 3                               # chunks per matmul weight-reuse block

_CACHE = {}


def _build_program():
    nc = bacc.Bacc("TRN2", target_bir_lowering=False, debug=False,
                   num_devices=NCORES)
    dt = mybir.dt

    x_d = nc.dram_tensor("x", [NPER, CIN, H, W], dt.int8, kind="ExternalInput")
    wt_d = nc.dram_tensor("wt", [TAPS, CIN, COUT], dt.int8, kind="ExternalInput")
    sc_d = nc.dram_tensor("scales", [COUT], dt.float32, kind="ExternalInput")
    zp_d = nc.dram_tensor("zp", [COUT], dt.int32, kind="ExternalInput")
    bi_d = nc.dram_tensor("bias", [COUT], dt.float32, kind="ExternalInput")
    out_d = nc.dram_tensor("out", [NPER, COUT, HO, WO], dt.int32,
                           kind="ExternalOutput")

    with TileContext(nc) as tc:
        with (
            tc.tile_pool(name="const", bufs=1) as cpool,
            tc.tile_pool(name="xin", bufs=2) as xpool,
            tc.tile_pool(name="xbf", bufs=2) as xbpool,
            tc.tile_pool(name="psum", bufs=2 * B_CHUNK, space="PSUM") as ppool,
            tc.tile_pool(name="tmp", bufs=4) as tpool,
            tc.tile_pool(name="outb", bufs=6) as opool,
        ):
            # ---- one-time constants ----
            # zeropoint row replicated to all 128 partitions via step-0 DMA
            zpb = cpool.tile([128, COUT], dt.int32)
            nc.sync.dma_start(out=zpb[:, :],
                              in_=zp_d[None, :].to_broadcast([128, COUT]))

            # PE warm-up: ~40 tiny matmuls fill the input-DMA wait right
            # after the boot barrier, flipping the HAM clock gate to 8/8
            # before the first real conv matmul issues.
            wupw = cpool.tile([1, 1], dt.bfloat16)
            nc.vector.memset(wupw[:, :], 1.0)
            wupx = cpool.tile([1, 128], dt.bfloat16)
            nc.vector.memset(wupx[:, :], 1.0)
            wups = ppool.tile([1, 128], dt.float32, name="wups", tag="wup",
                              bufs=1)
            for _ in range(40):
                nc.tensor.matmul(wups[:, :], wupw[:, :], wupx[:, :],
                                 start=True, stop=True)

            def load_image(n, pieces=1):
                # DMA + (x-7) bf16 conversion, split into `pieces` column
                # blocks per cin-tile so downstream matmuls (region-level
                # deps) can start before the whole image has landed.
                xi = xpool.tile([128, KT, XPAD], dt.int8, name="xi")
                xb = xbpool.tile([128, KT, XPAD], dt.bfloat16, name="xb")
                bnd = [0, 1680, HW] if pieces == 2 else [0, HW]
                for k in range(KT):
                    for p in range(len(bnd) - 1):
                        a, b = bnd[p], bnd[p + 1]
                        be = b if b < HW else XPAD  # convert pad cols too
                        nc.sync.dma_start(
                            out=xi[:, k, a:b],
                            in_=x_d[n, k * 128:(k + 1) * 128].rearrange(
                                "p h w -> p (h w)")[:, a:b])
                        # x' = x - 7, exact in bf16 (pad cols: finite garbage)
                        nc.vector.tensor_scalar(
                            xb[:, k, a:be], xi[:, k, a:be], -7.0,
                            None, mybir.AluOpType.add)
                return xb

            # ---- weights: int8 [tap, cin, cout] -> bf16 (w - zp) lhsT ----
            # Emission interleaves the first image's input load with the
            # weight DMA+subtracts in matmul consumption order (k-major),
            # so the first conv matmul fires as soon as tap (0,0) weights
            # and the first x columns have landed.  One weight DMA per
            # cin-tile (issue slots on the Sync queue cost ~620ns each).
            wi8 = cpool.tile([128, TAPS, KT, COUT], dt.int8)
            wb = cpool.tile([128, TAPS * KT, COUT], dt.bfloat16)

            xi0 = xpool.tile([128, KT, XPAD], dt.int8, name="xi")
            xb0 = xbpool.tile([128, KT, XPAD], dt.bfloat16, name="xb")
            XSPLIT = 1680  # covers chunk-block 0 reads (max 1626)

            def xdma0(k, a, b):
                nc.sync.dma_start(
                    out=xi0[:, k, a:b],
                    in_=x_d[0, k * 128:(k + 1) * 128].rearrange(
                        "p h w -> p (h w)")[:, a:b])

            def xconv0(k, a, b):
                nc.vector.tensor_scalar(xb0[:, k, a:b], xi0[:, k, a:b],
                                        -7.0, None, mybir.AluOpType.add)

            def wsub(k, t):
                nc.vector.tensor_tensor(
                    wb[:, t * KT + k, :], wi8[:, t, k, :], zpb[:, :],
                    mybir.AluOpType.subtract)

            xdma0(0, 0, XSPLIT)
            nc.sync.dma_start(
                out=wi8[:, 0:3, 0, :],
                in_=wt_d[0:3, 0:128, :].rearrange("t p o -> p t o"))
            nc.sync.dma_start(
                out=wi8[:, 3:TAPS, 0, :],
                in_=wt_d[3:TAPS, 0:128, :].rearrange("t p o -> p t o"))
            wsub(0, 0)
            xconv0(0, 0, XSPLIT)
            for t in range(1, TAPS):
                wsub(0, t)
            xdma0(0, XSPLIT, HW)
            xconv0(0, XSPLIT, XPAD)
            xdma0(1, 0, XSPLIT)
            xconv0(1, 0, XSPLIT)
            nc.sync.dma_start(
                out=wi8[:, :, 1, :],
                in_=wt_d[:, 128:256, :].rearrange("t p o -> p t o"))
            for t in range(TAPS):
                wsub(1, t)
            xdma0(1, XSPLIT, HW)
            xconv0(1, XSPLIT, XPAD)

            # combined output scale 0.01*scale[o] and bias, one column per m-tile
            sc2 = cpool.tile([128, MT], dt.float32)
            nc.sync.dma_start(out=sc2[:, :], in_=sc_d.rearrange("(m p) -> p m", p=128))
            nc.vector.tensor_scalar(sc2[:, :], sc2[:, :], 0.01, None,
                                    mybir.AluOpType.mult)
            bi2 = cpool.tile([128, MT], dt.float32)
            nc.sync.dma_start(out=bi2[:, :], in_=bi_d.rearrange("(m p) -> p m", p=128))

            # ---- per-image pipeline ----
            for n in range(NPER):
                xb = xb0 if n == 0 else load_image(n)

                for m in range(MT):
                    for cb in range(NCHUNK // B_CHUNK):
                        ps = [ppool.tile([128, CHUNK], dt.float32,
                                         name="ps", tag="ps")
                              for _ in range(B_CHUNK)]
                        # Final block runs chunk-major so per-chunk stops
                        # stagger and the tail epilogue overlaps the last
                        # matmuls (costs extra LDWEIGHTS, tail-only).
                        last_block = (n == NPER - 1 and m == MT - 1
                                      and cb == NCHUNK // B_CHUNK - 1)
                        def rhs_ap(k, c, dh, dw):
                            # 9 output rows x 54 valid cols of the shifted
                            # image: 2-level free AP (row stride 56) skips
                            # the 2 conv-overhang columns per row.
                            base = (9 * c + dh) * W + dw
                            return xb[:, k, base:base + 9 * W].rearrange(
                                "p (r w) -> p r w", w=W)[:, :, 0:WO]

                        if last_block:
                            for c0 in range(B_CHUNK):
                                c = cb * B_CHUNK + c0
                                first = True
                                for k in range(KT):
                                    for t in range(TAPS):
                                        dh, dw = t // KW, t % KW
                                        nc.tensor.matmul(
                                            ps[c0][:, :],
                                            wb[:, t * KT + k,
                                               m * 128:(m + 1) * 128],
                                            rhs_ap(k, c, dh, dw),
                                            start=first,
                                            stop=(k == KT - 1 and
                                                  t == TAPS - 1))
                                        first = False
                        else:
                            first = True
                            for k in range(KT):
                                for t in range(TAPS):
                                    dh, dw = t // KW, t % KW
                                    lhsT = wb[:, t * KT + k,
                                              m * 128:(m + 1) * 128]
                                    for c0 in range(B_CHUNK):
                                        c = cb * B_CHUNK + c0
                                        nc.tensor.matmul(
                                            ps[c0][:, :], lhsT,
                                            rhs_ap(k, c, dh, dw),
                                            start=first,
                                            stop=(k == KT - 1 and t == TAPS - 1))
                                    first = False
                        for c0 in range(B_CHUNK):
                            c = cb * B_CHUNK + c0
                            # y = 0.01*scale*S + bias   (fp32, per-partition)
                            tmp = tpool.tile([128, CHUNK], dt.float32)
                            nc.vector.tensor_scalar(
                                tmp[:, :], ps[c0][:, :],
                                sc2[:, m:m + 1], bi2[:, m:m + 1],
                                mybir.AluOpType.mult, mybir.AluOpType.add)
                            # round-to-nearest-even (psum already garbage-free)
                            t3 = tmp[:, :].rearrange("p (r w) -> p r w", w=WO)
                            ob = opool.tile([128, 9, WO], dt.int32)
                            nc.vector.tensor_scalar(
                                ob[:, :, :], t3[:, :, :], MAGIC, MAGIC,
                                mybir.AluOpType.add, mybir.AluOpType.subtract)
                            nc.sync.dma_start(
                                out=out_d[n, m * 128:(m + 1) * 128,
                                          9 * c:9 * (c + 1), :],
                                in_=ob[:, :, :])

    nc.compile()
    return nc


def kernel(**inputs) -> np.ndarray:
    x = np.ascontiguousarray(np.asarray(inputs["inputVec"], dtype=np.int8))
    w = np.asarray(inputs["weight"], dtype=np.int8)
    scales = np.ascontiguousarray(np.asarray(inputs["scales"], dtype=np.float32))
    zp = np.ascontiguousarray(np.asarray(inputs["zeropoints"], dtype=np.int32))
    bias = np.ascontiguousarray(np.asarray(inputs["bias"], dtype=np.float32))
    assert x.shape == (N, CIN, H, W) and w.shape == (COUT, CIN, KH, KW)

    # [cout, cin, kh, kw] -> [tap, cin, cout] so lhsT tiles DMA contiguously
    wt = np.ascontiguousarray(
        w.transpose(2, 3, 1, 0).reshape(TAPS, CIN, COUT))

    if "nc" not in _CACHE:
        _CACHE["nc"] = _build_program()
    nc = _CACHE["nc"]

    in_maps = [
        {"x": x[c * NPER:(c + 1) * NPER], "wt": wt, "scales": scales,
         "zp": zp, "bias": bias}
        for c in range(NCORES)
    ]
    res = run_bass_kernel_spmd(nc, in_maps, list(range(NCORES)))
    out = np.concatenate([res.results[c]["out"] for c in range(NCORES)], axis=0)
    return out

